# revision 10
# baseline (speedup 1.0000x reference)
"""Trainium2 Bass kernel for nn_Diagnet (S=1024, B=64, I=512, H=2048, O=512).

    u = einsum('sbi,hi->sbh', X, W_ih)
    h_{t} = |u_t + hh * h_{t-1}|   (scan over S, only final h needed)
    Y = h_final @ W_ho.T + b_ho

Strategy (8 NeuronCores, data-parallel over batch, 8 batch rows per core):

* H lanes are permuted so hh is sorted descending and split into 16
  chunks of 128.  The recurrence is a contraction with per-lane factor
  a=hh<1, so a chunk whose largest a satisfies a^K < 1e-10 only needs
  the last K steps: the input->hidden GEMM and the scan skip everything
  earlier (this is exact to ~1e-10 relative, far below fp32 noise).
* Within each 64-step block the state is kept pre-scaled as
  m_tau = a^(63-tau) * h.  Then the step is a multiply-free
  m = |m + a^(63-tau) u_t|, applied by a custom fused DVE op
  (out = |in0 + in1|), one instruction per step over all active chunks.
  Entering a block multiplies the state once by a^64.  Scales a^(63-tau)
  fold into the PSUM->SBUF move of the GEMM output (one tensor_tensor
  multiply).  Underflow of a^64 for small-a lanes reproduces the
  truncation automatically, and no overflow is possible (scales <= 1).
* GEMM: X rows are loaded (b,s)-ordered, transposed 128x128 on the PE,
  and multiplied against pre-transposed (host-side) W_ih^T in fp32.
  PSUM layout [h, (b,t)] hands each scan step a contiguous slice.
* Final projection: h_final tiles (already [h,b] on chip) are the
  stationary operand against host-transposed W_ho^T; bias added on DVE.
"""

import math
import os

from contextlib import ExitStack

import numpy as np

S, B, I, H, O = 1024, 64, 512, 2048, 512
NCORES = 8
BC = B // NCORES  # 8 batch rows per core
TB = 64  # time block == scan window
NBLK = S // TB  # 16
NCH = H // 128  # 16 h-chunks
LN_TRUNC = 23.03  # a^K <= e^-23 ~ 1e-10 -> truncate

_CACHE = {}


def _register_abs_add():
    import concourse.dve_ops as dve_ops
    from concourse.dve_spec import Spec, Src0, Src1, Zero, maxx, lower
    from concourse.dve_uop import DveOpSpec

    for op in dve_ops.OPS:
        if op.name == "ABS_ADD_ANT":
            return op
    x = Src0 + Src1
    spec = Spec(
        body=maxx(x, Zero - x),
        reference=lambda in0, in1, s0, s1, imm2: np.abs(
            in0.astype(np.float32) + in1.astype(np.float32)
        ),
    )
    row = max(dve_ops._SUB_OPCODE_FOR_NAME.values()) + 1
    assert row < 0x20
    shas = {}
    for ver in ("v3", "v4"):
        s = DveOpSpec(name="ABS_ADD_ANT", opcode=row, uops=lower(spec, ver=ver), rd1_en=True)
        shas[ver] = s.sha(ver)
    op = dve_ops.DveOp("ABS_ADD_ANT", spec, subdim=False, uops_sha=shas)
    dve_ops._SUB_OPCODE_FOR_NAME["ABS_ADD_ANT"] = row
    dve_ops.OPS.append(op)
    dve_ops.CUSTOM_DVE_SPECS["ABS_ADD_ANT"] = spec
    return op


def _make_plan(hh):
    a = np.maximum(np.abs(hh.astype(np.float64)), 1e-30)
    # jax uniform is [0,1); abs is a no-op safeguard.
    perm = np.argsort(-a, kind="stable")
    a_s = a[perm]
    first_block = []
    for g in range(NCH):
        amax = a_s[g * 128]
        if amax >= math.exp(-LN_TRUNC / S):
            kg = S
        else:
            kg = min(S, int(math.ceil(LN_TRUNC / math.log(1.0 / amax))))
        kg = min(S, ((kg + TB - 1) // TB) * TB)
        first_block.append(NBLK - kg // TB)
    # chunks sorted by a desc -> first_block nondecreasing -> active set is
    # always a chunk prefix.
    assert all(
        first_block[g] <= first_block[g + 1] for g in range(NCH - 1)
    ), first_block
    ag = a_s.reshape(NCH, 128).T  # [128, NCH] lane a per chunk
    tau = np.arange(TB)
    sc = ag[:, :, None] ** (TB - 1 - tau)[None, None, :]  # [128, NCH, TB]
    a64 = np.repeat(ag**TB, BC, axis=1)  # [128, NCH*BC]
    return {
        "perm": perm,
        "first_block": tuple(first_block),
        "SC": sc.reshape(128, NCH * TB).astype(np.float32),
        "A64": a64.astype(np.float32),
    }


def _build(first_block, use_f32r):
    import concourse.mybir as mybir
    import concourse.tile as tile
    from concourse import bacc
    from concourse.bass import ds

    ABS_ADD = _register_abs_add()
    f32 = mybir.dt.float32
    gemm_dt = mybir.dt.float32r if use_f32r else f32

    nc = bacc.Bacc("TRN2", target_bir_lowering=False, debug=False, num_devices=NCORES)
    X = nc.dram_tensor("X", [NBLK, I // 128, 128, TB * BC], gemm_dt, kind="ExternalInput").ap()
    WIHT = nc.dram_tensor("WIHT", [I, H], gemm_dt, kind="ExternalInput").ap()
    WHOT = nc.dram_tensor("WHOT", [H, O], f32, kind="ExternalInput").ap()
    BIAS = nc.dram_tensor("BIAS", [BC, O], f32, kind="ExternalInput").ap()
    SC = nc.dram_tensor("SC", [128, NCH * TB], f32, kind="ExternalInput").ap()
    A64 = nc.dram_tensor("A64", [128, NCH * BC], f32, kind="ExternalInput").ap()
    Y = nc.dram_tensor("Y", [BC, O], f32, kind="ExternalOutput").ap()

    NI = I // 128  # 4 i-chunks

    with tile.TileContext(nc) as tc:
        with ExitStack() as ctx:
            consts = ctx.enter_context(tc.tile_pool(name="consts", bufs=1))
            xtpool = ctx.enter_context(tc.tile_pool(name="xt", bufs=3))
            upool = ctx.enter_context(tc.tile_pool(name="ubuf", bufs=1))
            ypool = ctx.enter_context(tc.tile_pool(name="yout", bufs=1))
            gpool = ctx.enter_context(tc.tile_pool(name="gpsum", bufs=4, space="PSUM"))
            fpool = ctx.enter_context(tc.tile_pool(name="fpsum", bufs=1, space="PSUM"))

            # constants
            wiht = [consts.tile([128, H], gemm_dt, tag=f"wiht{ic}", name=f"wiht{ic}") for ic in range(NI)]
            for ic in range(NI):
                nc.sync.dma_start(wiht[ic][:], WIHT[ds(ic * 128, 128), :])
            sc_t = consts.tile([128, NCH * TB], f32, tag="sc", name="sc_t")
            nc.sync.dma_start(sc_t[:], SC)
            a64_t = consts.tile([128, NCH * BC], f32, tag="a64", name="a64_t")
            nc.sync.dma_start(a64_t[:], A64)
            m_t = consts.tile([128, NCH * BC], f32, tag="state", name="m_t")
            nc.vector.memset(m_t[:], 0.0)

            acts = [sum(1 for fb in first_block if fb <= kb) for kb in range(NBLK)]
            assert all(a >= 1 for a in acts)
            u_tiles = [None] * NBLK

            def produce(kb):
                act = acts[kb]
                # --- load pre-transposed X tiles [i, (b,t)] ---
                xt = []
                for ic in range(NI):
                    xt_ic = xtpool.tile([128, TB * BC], gemm_dt, tag=f"xt{ic}", name=f"xt_{kb}_{ic}")
                    nc.sync.dma_start(xt_ic[:], X[kb, ic])
                    xt.append(xt_ic)
                # u buffer for this block: [128, (tau, active-chunk, b)]
                u_t = upool.tile([128, TB * act * BC], f32, tag=f"u{kb}", name=f"u_{kb}")
                u_tiles[kb] = u_t
                for g in range(act):
                    ps = gpool.tile([128, TB * BC], f32, tag="gp", name=f"gp_{kb}_{g}")
                    for ic in range(NI):
                        nc.tensor.matmul(
                            ps[:],
                            wiht[ic][:, ds(g * 128, 128)],
                            xt[ic][:],
                            start=(ic == 0),
                            stop=(ic == NI - 1),
                        )
                    # scaled move psum->sbuf:
                    # u_t[p, tau*act*BC + g*BC + b] = ps[p, b*TB+tau]*SC[p,g*TB+tau]
                    dst = u_t[:].rearrange("p (t c) -> p t c", t=TB)[
                        :, :, ds(g * BC, BC)
                    ]
                    srcp = ps[:].rearrange("p (b t) -> p t b", b=BC)
                    scl = sc_t[:, ds(g * TB, TB)].broadcast_to([128, TB, BC])
                    nc.vector.tensor_tensor(dst, srcp, scl, mybir.AluOpType.mult)

            def scan(kb):
                act = acts[kb]
                na = act * BC
                u_t = u_tiles[kb]
                nc.gpsimd.tensor_tensor(
                    m_t[:, 0:na], m_t[:, 0:na], a64_t[:, 0:na], mybir.AluOpType.mult
                )
                for tau in range(TB):
                    nc.vector._custom_dve(
                        ABS_ADD,
                        out=m_t[:, 0:na],
                        in0=m_t[:, 0:na],
                        in1=u_t[:, ds(tau * act * BC, na)],
                    )

            LAG = 2
            for kb in range(NBLK):
                produce(kb)
                if kb >= LAG:
                    scan(kb - LAG)
            for kb in range(NBLK - LAG, NBLK):
                scan(kb)

            # --- final projection: Y = h^T @ WHOT + bias ---
            whot = [consts.tile([128, O], f32, tag=f"whot{g}", name=f"whot{g}") for g in range(NCH)]
            for g in range(NCH):
                nc.sync.dma_start(whot[g][:], WHOT[ds(g * 128, 128), :])
            bias_t = ypool.tile([BC, O], f32, tag="bias", name="bias_t")
            nc.sync.dma_start(bias_t[:], BIAS)
            psy = fpool.tile([BC, O], f32, tag="fy", name="psy")
            for g in range(NCH):
                nc.tensor.matmul(
                    psy[:],
                    m_t[:, ds(g * BC, BC)],
                    whot[g][:],
                    start=(g == 0),
                    stop=(g == NCH - 1),
                )
            y_t = ypool.tile([BC, O], f32, tag="y", name="y_t")
            nc.vector.tensor_tensor(y_t[:], psy[:], bias_t[:], mybir.AluOpType.add)
            nc.sync.dma_start(Y, y_t[:])
    nc.compile()
    return nc


def _get_program(first_block, use_f32r):
    key = (first_block, use_f32r)
    if key not in _CACHE:
        _CACHE[key] = _build(first_block, use_f32r)
    return _CACHE[key]


def _round_f32r(x):
    """Round fp32 array to fp32r (s8e11) representable values."""
    u = np.ascontiguousarray(x).view(np.uint32)
    r = ((u.astype(np.uint64) + 0x800) & 0xFFFFF000).astype(np.uint32)
    return r.view(np.float32).reshape(x.shape)


def _ensure_ntff_hook():
    """Provide antenv.axon_hooks (absent in this image) so trace=True works."""
    import sys
    import types

    if "antenv.axon_hooks" in sys.modules:
        return True
    try:
        import antenv

        mod = types.ModuleType("antenv.axon_hooks")
        mod._hook = None

        def set_axon_ntff_profile_hook(h):
            mod._hook = h

        def get_axon_ntff_profile_hook():
            return mod._hook

        mod.set_axon_ntff_profile_hook = set_axon_ntff_profile_hook
        mod.get_axon_ntff_profile_hook = get_axon_ntff_profile_hook
        sys.modules["antenv.axon_hooks"] = mod
        antenv.axon_hooks = mod

        from trn_agent_boot.trn_boot import _ntff_profile_via_ctypes

        hook = _ntff_profile_via_ctypes("/opt/axon/libaxon_pjrt.so")
        mod.set_axon_ntff_profile_hook(hook)
        return hook is not None
    except Exception:
        return False


def kernel(X, W_ih, hh, W_ho, b_ho):
    from concourse import bass_utils

    X = np.asarray(X, dtype=np.float32)
    W_ih = np.asarray(W_ih, dtype=np.float32)
    hh = np.asarray(hh, dtype=np.float32)
    W_ho = np.asarray(W_ho, dtype=np.float32)
    b_ho = np.asarray(b_ho, dtype=np.float32)

    use_f32r = bool(int(os.environ.get("DIAG_F32R", "0")))
    plan = _make_plan(hh)
    perm = plan["perm"]
    nc = _get_program(plan["first_block"], use_f32r)

    wiht = np.ascontiguousarray(W_ih[perm].T)  # [I, H]
    if use_f32r:
        wiht = _round_f32r(wiht)
    whot = np.ascontiguousarray(W_ho[:, perm].T)  # [H, O]
    bias = np.tile(b_ho[None, :], (BC, 1)).astype(np.float32)

    common = {
        "WIHT": wiht,
        "WHOT": whot,
        "BIAS": bias,
        "SC": plan["SC"],
        "A64": plan["A64"],
    }
    in_maps = []
    for m in range(NCORES):
        im = dict(common)
        xm = X[:, m * BC : (m + 1) * BC, :]  # [S, BC, I]
        # device tile layout [NBLK, NI, 128(i), (b, tau)]
        xt = xm.transpose(2, 1, 0).reshape(I // 128, 128, BC, NBLK, TB)
        xt = np.ascontiguousarray(xt.transpose(3, 0, 1, 2, 4)).reshape(
            NBLK, I // 128, 128, TB * BC
        )
        if use_f32r:
            xt = _round_f32r(xt)
        im["X"] = xt
        in_maps.append(im)

    trace = bool(int(os.environ.get("DIAG_TRACE", "0")))
    if trace:
        trace = _ensure_ntff_hook()
    res = bass_utils.run_bass_kernel_spmd(
        nc,
        in_maps,
        core_ids=list(range(NCORES)),
        trace=trace,
        tmpdir=os.environ.get("DIAG_TRACE_DIR") or None,
    )
    if res.exec_time_ns is not None:
        kernel.last_exec_time_ns = res.exec_time_ns
        kernel.last_mean_exec_time_ns = res.mean_exec_time_ns
    Yfull = np.concatenate([r["Y"] for r in res.results], axis=0)
    return Yfull


kernel.last_exec_time_ns = None
kernel.last_mean_exec_time_ns = None
